# revision 4
# baseline (speedup 1.0000x reference)
"""Multi-head causal attention (B=4, T=2048, D=1024, H=16, Dh=64) on 8 trn2 cores.

Sharding: core c -> (batch b = c//2, head-group g = c%2 covering heads 8g..8g+7).
Each core computes QKV projection for its batch/head-group, causal attention,
and a partial output projection over its 512 head-dims.  Host sums the two
partial outputs per batch.

Device-side dataflow per core (all matmul inputs bf16, accumulation fp32):
  xT [1024, 2048]  (x[b] transposed on host)
  qT, kT [512, 2048]   (projection emitted transposed: heads stacked on partitions,
                        pairs of heads per 128-partition block)
  v  [2048, 8, 65]     (natural layout, 65th column = 1.0 so the PV matmul also
                        produces the softmax denominator as output row 64)
  S^T [k, q] blocks via PE row-tiling: two K=64 head-matmuls run concurrently
  P = exp(S^T) on ScalarE (no max subtraction: |scores| <~ 8 by construction)
  causal: strictly-upper blocks skipped, diagonal 128x128 blocks masked with a
  multiplicative triu constant on VectorE
  out^T_aug [65, q] = v_aug^T @ P^T accumulated in PSUM; row 64 = sum(exp)
  normalize: reciprocal of row 64, DMA-broadcast across 64 partitions, multiply
  out-projection: OT [512, 2048] as lhsT against Wout shard, fp32 partial out
"""

import sys

if "/opt/trn_rl_repo" not in sys.path:
    sys.path.insert(0, "/opt/trn_rl_repo")

from contextlib import ExitStack

import ml_dtypes
import numpy as np

import concourse.bass as bass
import concourse.bacc as bacc
import concourse.mybir as mybir
import concourse.tile as tile
from concourse.bass_utils import run_bass_kernel_spmd

BF16 = mybir.dt.bfloat16
F32 = mybir.dt.float32
NPBF16 = ml_dtypes.bfloat16

B, T, D = 4, 2048, 1024
H, DH = 16, 64
HPG = 8          # heads per group (per core)
GD = HPG * DH    # 512 head-dims per core
NT = T // 128    # 16 t-blocks
NC = D // 128    # 8 model-dim chunks
NQ = T // 512    # 4 q-spans
SCALE = 1.0 / 8.0


def build_attention_kernel(mode: str = "causal"):
    """mode: 'causal' (tril mask), 'dense' (all-ones mask), 'masked' (arbitrary)."""
    nc = bacc.Bacc("TRN2", target_bir_lowering=False)

    xT_d = nc.dram_tensor("xT", [D, T], BF16, kind="ExternalInput")
    wq_d = nc.dram_tensor("wq", [D, GD], BF16, kind="ExternalInput")
    wk_d = nc.dram_tensor("wk", [D, GD], BF16, kind="ExternalInput")
    wv_d = nc.dram_tensor("wv", [D, GD], BF16, kind="ExternalInput")
    wo_d = nc.dram_tensor("wout", [GD, D], BF16, kind="ExternalInput")
    triu_d = nc.dram_tensor("triu", [128, 128], BF16, kind="ExternalInput")
    if mode == "masked":
        m01_d = nc.dram_tensor("m01T", [T, T], BF16, kind="ExternalInput")
    out_d = nc.dram_tensor("out", [T, D], F32, kind="ExternalOutput")

    with tile.TileContext(nc) as tc, ExitStack() as ctx:
        const = ctx.enter_context(tc.tile_pool(name="const", bufs=1))
        ppool = ctx.enter_context(tc.tile_pool(name="ppool", bufs=6))
        rpool = ctx.enter_context(tc.tile_pool(name="rpool", bufs=4))
        bpool = ctx.enter_context(tc.tile_pool(name="bpool", bufs=4))
        opool = ctx.enter_context(tc.tile_pool(name="opool", bufs=4))
        mpool = ctx.enter_context(tc.tile_pool(name="mpool", bufs=4))
        ps = ctx.enter_context(
            tc.tile_pool(name="ps", bufs=8, space=bass.MemorySpace.PSUM)
        )

        # ---- load inputs --------------------------------------------------
        xT = const.tile([128, NC, T], BF16)
        wq = const.tile([128, NC, GD], BF16)
        wk = const.tile([128, NC, GD], BF16)
        wv = const.tile([128, NC, GD], BF16)
        for c in range(NC):
            nc.sync.dma_start(xT[:, c, :], xT_d[c * 128 : (c + 1) * 128, :])
            nc.sync.dma_start(wq[:, c, :], wq_d[c * 128 : (c + 1) * 128, :])
            nc.sync.dma_start(wk[:, c, :], wk_d[c * 128 : (c + 1) * 128, :])
            nc.sync.dma_start(wv[:, c, :], wv_d[c * 128 : (c + 1) * 128, :])
        wo = const.tile([128, GD // 128, D], BF16)
        for c in range(GD // 128):
            nc.sync.dma_start(wo[:, c, :], wo_d[c * 128 : (c + 1) * 128, :])
        triu = const.tile([128, 128], BF16)
        nc.sync.dma_start(triu[:], triu_d[:])

        qT = const.tile([128, GD // 128, T], BF16)
        kT = const.tile([128, GD // 128, T], BF16)
        v = const.tile([128, NT, HPG, DH + 1], BF16)
        ot = const.tile([128, GD // 128, T], BF16)

        nc.vector.memset(v[:, :, :, DH : DH + 1], 1.0)

        # ---- qkv projection ----------------------------------------------
        def proj_qk(dst, w, j):
            # dst[:, j, :] <- (w[:, j*128:(j+1)*128]).T @ xT   -> [128, T]
            acc = [ps.tile([128, 512], F32, tag="ps", name=f"acc{n}") for n in range(NQ)]
            for c in range(NC):
                lhsT = w[:, c, j * 128 : (j + 1) * 128]
                for n in range(NQ):
                    nc.tensor.matmul(
                        acc[n][:],
                        lhsT,
                        xT[:, c, n * 512 : (n + 1) * 512],
                        start=(c == 0),
                        stop=(c == NC - 1),
                    )
            for n in range(NQ):
                nc.vector.tensor_copy(dst[:, j, n * 512 : (n + 1) * 512], acc[n][:])

        def proj_v(t):
            acc = ps.tile([128, 512], F32, tag="ps", name="accv")
            for c in range(NC):
                nc.tensor.matmul(
                    acc[:],
                    xT[:, c, t * 128 : (t + 1) * 128],
                    wv[:, c, :],
                    start=(c == 0),
                    stop=(c == NC - 1),
                )
            nc.vector.tensor_copy(
                v[:, t, :, 0:DH], acc[:].rearrange("p (h e) -> p h e", e=DH)
            )

        proj_qk(qT, wq, 0)
        proj_qk(kT, wk, 0)
        for t in range(NT):
            proj_v(t)
        for j in range(1, GD // 128):
            proj_qk(qT, wq, j)
            proj_qk(kT, wk, j)

        # ---- attention ----------------------------------------------------
        for j in range(GD // 128):  # head pair
            for qi in range(NQ):  # 512-wide q span
                q0 = qi * 512
                nki = 4 * qi + 4 if mode == "causal" else NT
                pv = [ps.tile([128, 512], F32, tag="ps", name=f"pv{h}") for h in range(2)]
                for ki in range(nki):
                    d = ki - 4 * qi  # >=0: diagonal band (causal mode only)
                    lo = max(d, 0) * 128 if mode == "causal" else 0
                    if mode == "masked":
                        m01 = mpool.tile([128, 512], BF16, tag="m01")
                        nc.sync.dma_start(
                            m01[:],
                            m01_d[ki * 128 : (ki + 1) * 128, q0 : q0 + 512],
                        )
                    ptiles = []
                    for hh in range(2):
                        s = ps.tile([128, 512], F32, tag="ps", name="s")
                        nc.tensor.matmul(
                            s[:, lo:512],
                            kT[hh * 64 : (hh + 1) * 64, j, ki * 128 : (ki + 1) * 128],
                            qT[hh * 64 : (hh + 1) * 64, j, q0 + lo : q0 + 512],
                            start=True,
                            stop=True,
                        )
                        p = ppool.tile([128, 512], BF16, tag="p")
                        nc.scalar.activation(
                            p[:, lo:512],
                            s[:, lo:512],
                            mybir.ActivationFunctionType.Exp,
                        )
                        if mode == "causal" and 0 <= d:
                            nc.vector.tensor_mul(
                                p[:, lo : lo + 128], p[:, lo : lo + 128], triu[:]
                            )
                        elif mode == "masked":
                            nc.vector.tensor_mul(p[:], p[:], m01[:])
                        ptiles.append(p)
                    for hh in range(2):
                        nc.tensor.matmul(
                            pv[hh][0:65, lo:512],
                            v[:, ki, 2 * j + hh, :],
                            ptiles[hh][:, lo:512],
                            start=(ki == 0),
                            stop=(ki == nki - 1),
                        )
                # normalize and store into ot
                for hh in range(2):
                    recip = rpool.tile([1, 512], F32, tag="recip")
                    nc.vector.reciprocal(recip[:], pv[hh][64:65, :])
                    bc = bpool.tile([64, 512], F32, tag="bc")
                    nc.gpsimd.partition_broadcast(bc[:], recip[:])
                    if hh == 0:
                        nc.vector.tensor_mul(
                            ot[0:64, j, q0 : q0 + 512], pv[hh][0:64, :], bc[:]
                        )
                    else:
                        otn = opool.tile([64, 512], BF16, tag="otn")
                        nc.vector.tensor_mul(otn[:], pv[hh][0:64, :], bc[:])
                        nc.sync.dma_start(ot[64:128, j, q0 : q0 + 512], otn[:])

        # ---- output projection -------------------------------------------
        for t in range(NT):
            for n2 in range(2):
                acc = ps.tile([128, 512], F32, tag="ps", name="acco")
                for c in range(GD // 128):
                    nc.tensor.matmul(
                        acc[:],
                        ot[:, c, t * 128 : (t + 1) * 128],
                        wo[:, c, n2 * 512 : (n2 + 1) * 512],
                        start=(c == 0),
                        stop=(c == GD // 128 - 1),
                    )
                ob = opool.tile([128, 512], F32, tag="ob")
                nc.vector.tensor_copy(ob[:], acc[:])
                nc.sync.dma_start(
                    out_d[t * 128 : (t + 1) * 128, n2 * 512 : (n2 + 1) * 512], ob[:]
                )

    nc.compile()
    return nc


_NC_CACHE: dict = {}


def _get_kernel(mode: str):
    if mode not in _NC_CACHE:
        _NC_CACHE[mode] = build_attention_kernel(mode)
    return _NC_CACHE[mode]


def make_in_maps(x, mask, Wqkv, Wout):
    tril = np.tril(np.ones((T, T), dtype=np.int32))
    m = np.asarray(mask[0, 0])
    if np.array_equal(m, tril):
        mode = "causal"
    elif np.all(m == 1):
        mode = "dense"
    else:
        mode = "masked"

    triu128 = np.triu(np.ones((128, 128), dtype=np.float32)).astype(NPBF16)
    in_maps = []
    for c in range(8):
        b, g = c // 2, c % 2
        im = {
            "xT": np.ascontiguousarray(x[b].T).astype(NPBF16),
            "wq": Wqkv[:, g * GD : (g + 1) * GD].astype(NPBF16),
            "wk": (Wqkv[:, D + g * GD : D + (g + 1) * GD] * SCALE).astype(NPBF16),
            "wv": Wqkv[:, 2 * D + g * GD : 2 * D + (g + 1) * GD].astype(NPBF16),
            "wout": Wout[g * GD : (g + 1) * GD, :].astype(NPBF16),
            "triu": triu128,
        }
        if mode == "masked":
            im["m01T"] = np.ascontiguousarray(m.T).astype(NPBF16)
        in_maps.append(im)
    return mode, in_maps


def kernel(x, mask, Wqkv, Wout):
    x = np.asarray(x)
    mask = np.asarray(mask)
    Wqkv = np.asarray(Wqkv)
    Wout = np.asarray(Wout)
    mode, in_maps = make_in_maps(x, mask, Wqkv, Wout)
    nc = _get_kernel(mode)
    res = run_bass_kernel_spmd(nc, in_maps, core_ids=list(range(8)))
    out = np.zeros((B, T, D), dtype=np.float32)
    for c in range(8):
        out[c // 2] += res.results[c]["out"]
    return out


# revision 6
# speedup vs baseline: 91.4632x; 91.4632x over previous
"""Multi-head causal attention (B=4, T=2048, D=1024, H=16, Dh=64) on 8 trn2 cores.

Sharding: core c -> (batch b = c//2, head-group g = c%2 covering heads 8g..8g+7).
Each core computes QKV projection for its batch/head-group, causal attention,
and a partial output projection over its 512 head-dims.  Host sums the two
partial outputs per batch.

Device-side dataflow per core (all matmul inputs bf16, accumulation fp32):
  xT [1024, 2048]  (x[b] transposed on host)
  qT, kT [512, 2048]   (projection emitted transposed: heads stacked on partitions,
                        pairs of heads per 128-partition block)
  v  [2048, 8, 65]     (natural layout, 65th column = 1.0 so the PV matmul also
                        produces the softmax denominator as output row 64)
  S^T [k, q] blocks via PE row-tiling: two K=64 head-matmuls run concurrently
  P = exp(S^T) on ScalarE (no max subtraction: |scores| <~ 8 by construction)
  causal: strictly-upper blocks skipped, diagonal 128x128 blocks masked with a
  multiplicative triu constant on VectorE
  out^T_aug [65, q] = v_aug^T @ P^T accumulated in PSUM; row 64 = sum(exp)
  normalize: reciprocal of row 64, DMA-broadcast across 64 partitions, multiply
  out-projection: OT [512, 2048] as lhsT against Wout shard, fp32 partial out
"""

import sys

if "/opt/trn_rl_repo" not in sys.path:
    sys.path.insert(0, "/opt/trn_rl_repo")

from contextlib import ExitStack

import ml_dtypes
import numpy as np

import concourse.bass as bass
import concourse.bacc as bacc
import concourse.mybir as mybir
import concourse.tile as tile
from concourse.bass_utils import run_bass_kernel_spmd

BF16 = mybir.dt.bfloat16
F32 = mybir.dt.float32
NPBF16 = ml_dtypes.bfloat16

B, T, D = 4, 2048, 1024
H, DH = 16, 64
HPG = 8          # heads per group (per core)
GD = HPG * DH    # 512 head-dims per core
NT = T // 128    # 16 t-blocks
NC = D // 128    # 8 model-dim chunks
NQ = T // 512    # 4 q-spans
SCALE = 1.0 / 8.0


def build_attention_kernel(mode: str = "causal", reps: int = 1):
    """mode: 'causal' (tril mask), 'dense' (all-ones mask), 'masked' (arbitrary).

    reps > 1 wraps the compute body in a hardware For_i loop (for timing the
    kernel body without host dispatch overhead)."""
    nc = bacc.Bacc("TRN2", target_bir_lowering=False)

    xT_d = nc.dram_tensor("xT", [D, T], BF16, kind="ExternalInput")
    wq_d = nc.dram_tensor("wq", [D, GD], BF16, kind="ExternalInput")
    wk_d = nc.dram_tensor("wk", [D, GD], BF16, kind="ExternalInput")
    wv_d = nc.dram_tensor("wv", [D, GD], BF16, kind="ExternalInput")
    wo_d = nc.dram_tensor("wout", [GD, D], BF16, kind="ExternalInput")
    triu_d = nc.dram_tensor("triu", [128, 128], BF16, kind="ExternalInput")
    if mode == "masked":
        m01_d = nc.dram_tensor("m01T", [T, T], BF16, kind="ExternalInput")
    out_d = nc.dram_tensor("out", [T, D], F32, kind="ExternalOutput")

    with tile.TileContext(nc) as tc, ExitStack() as ctx:
        const = ctx.enter_context(tc.tile_pool(name="const", bufs=1))
        ppool = ctx.enter_context(tc.tile_pool(name="ppool", bufs=6))
        rpool = ctx.enter_context(tc.tile_pool(name="rpool", bufs=4))
        bpool = ctx.enter_context(tc.tile_pool(name="bpool", bufs=4))
        opool = ctx.enter_context(tc.tile_pool(name="opool", bufs=4))
        mpool = ctx.enter_context(tc.tile_pool(name="mpool", bufs=4))
        ps = ctx.enter_context(
            tc.tile_pool(name="ps", bufs=8, space=bass.MemorySpace.PSUM)
        )

        # ---- load inputs --------------------------------------------------
        xT = const.tile([128, NC, T], BF16)
        wq = const.tile([128, NC, GD], BF16)
        wk = const.tile([128, NC, GD], BF16)
        wv = const.tile([128, NC, GD], BF16)
        for c in range(NC):
            nc.sync.dma_start(xT[:, c, :], xT_d[c * 128 : (c + 1) * 128, :])
            nc.sync.dma_start(wq[:, c, :], wq_d[c * 128 : (c + 1) * 128, :])
            nc.sync.dma_start(wk[:, c, :], wk_d[c * 128 : (c + 1) * 128, :])
            nc.sync.dma_start(wv[:, c, :], wv_d[c * 128 : (c + 1) * 128, :])
        wo = const.tile([128, GD // 128, D], BF16)
        for c in range(GD // 128):
            nc.sync.dma_start(wo[:, c, :], wo_d[c * 128 : (c + 1) * 128, :])
        triu = const.tile([128, 128], BF16)
        nc.sync.dma_start(triu[:], triu_d[:])

        qT = const.tile([128, GD // 128, T], BF16)
        kT = const.tile([128, GD // 128, T], BF16)
        v = const.tile([128, NT, HPG, DH + 1], BF16)
        ot = const.tile([128, GD // 128, T], BF16)

        nc.vector.memset(v[:, :, :, DH : DH + 1], 1.0)

        def body():
            _body(nc, tc, mode, ps, ppool, rpool, bpool, opool, mpool,
                  xT, wq, wk, wv, wo, triu, qT, kT, v, ot,
                  m01_d if mode == "masked" else None, out_d)

        if reps > 1:
            with tc.For_i(0, reps, 1):
                body()
        else:
            body()

    nc.compile()
    return nc


def _body(nc, tc, mode, ps, ppool, rpool, bpool, opool, mpool,
          xT, wq, wk, wv, wo, triu, qT, kT, v, ot, m01_d, out_d):
    if True:
        # ---- qkv projection ----------------------------------------------
        def proj_qk(dst, w, j):
            # dst[:, j, :] <- (w[:, j*128:(j+1)*128]).T @ xT   -> [128, T]
            acc = [ps.tile([128, 512], F32, tag="ps", name=f"acc{n}") for n in range(NQ)]
            for c in range(NC):
                lhsT = w[:, c, j * 128 : (j + 1) * 128]
                for n in range(NQ):
                    nc.tensor.matmul(
                        acc[n][:],
                        lhsT,
                        xT[:, c, n * 512 : (n + 1) * 512],
                        start=(c == 0),
                        stop=(c == NC - 1),
                    )
            for n in range(NQ):
                nc.vector.tensor_copy(dst[:, j, n * 512 : (n + 1) * 512], acc[n][:])

        def proj_v(t):
            acc = ps.tile([128, 512], F32, tag="ps", name="accv")
            for c in range(NC):
                nc.tensor.matmul(
                    acc[:],
                    xT[:, c, t * 128 : (t + 1) * 128],
                    wv[:, c, :],
                    start=(c == 0),
                    stop=(c == NC - 1),
                )
            nc.vector.tensor_copy(
                v[:, t, :, 0:DH], acc[:].rearrange("p (h e) -> p h e", e=DH)
            )

        proj_qk(qT, wq, 0)
        proj_qk(kT, wk, 0)
        for t in range(NT):
            proj_v(t)
        for j in range(1, GD // 128):
            proj_qk(qT, wq, j)
            proj_qk(kT, wk, j)

        # ---- attention ----------------------------------------------------
        for j in range(GD // 128):  # head pair
            for qi in range(NQ):  # 512-wide q span
                q0 = qi * 512
                nki = 4 * qi + 4 if mode == "causal" else NT
                pv = [ps.tile([128, 512], F32, tag="ps", name=f"pv{h}") for h in range(2)]
                for ki in range(nki):
                    d = ki - 4 * qi  # >=0: diagonal band (causal mode only)
                    lo = max(d, 0) * 128 if mode == "causal" else 0
                    if mode == "masked":
                        m01 = mpool.tile([128, 512], BF16, tag="m01")
                        nc.sync.dma_start(
                            m01[:],
                            m01_d[ki * 128 : (ki + 1) * 128, q0 : q0 + 512],
                        )
                    ptiles = []
                    for hh in range(2):
                        s = ps.tile([128, 512], F32, tag="ps", name="s")
                        nc.tensor.matmul(
                            s[:, lo:512],
                            kT[hh * 64 : (hh + 1) * 64, j, ki * 128 : (ki + 1) * 128],
                            qT[hh * 64 : (hh + 1) * 64, j, q0 + lo : q0 + 512],
                            start=True,
                            stop=True,
                        )
                        p = ppool.tile([128, 512], BF16, tag="p")
                        nc.scalar.activation(
                            p[:, lo:512],
                            s[:, lo:512],
                            mybir.ActivationFunctionType.Exp,
                        )
                        if mode == "causal" and 0 <= d:
                            nc.vector.tensor_mul(
                                p[:, lo : lo + 128], p[:, lo : lo + 128], triu[:]
                            )
                        elif mode == "masked":
                            nc.vector.tensor_mul(p[:], p[:], m01[:])
                        ptiles.append(p)
                    for hh in range(2):
                        nc.tensor.matmul(
                            pv[hh][0:65, lo:512],
                            v[:, ki, 2 * j + hh, :],
                            ptiles[hh][:, lo:512],
                            start=(ki == 0),
                            stop=(ki == nki - 1),
                        )
                # normalize and store into ot
                for hh in range(2):
                    recip = rpool.tile([1, 512], F32, tag="recip")
                    nc.vector.reciprocal(recip[:], pv[hh][64:65, :])
                    bc = bpool.tile([64, 512], F32, tag="bc")
                    nc.gpsimd.partition_broadcast(bc[:], recip[:])
                    if hh == 0:
                        nc.vector.tensor_mul(
                            ot[0:64, j, q0 : q0 + 512], pv[hh][0:64, :], bc[:]
                        )
                    else:
                        otn = opool.tile([64, 512], BF16, tag="otn")
                        nc.vector.tensor_mul(otn[:], pv[hh][0:64, :], bc[:])
                        nc.sync.dma_start(ot[64:128, j, q0 : q0 + 512], otn[:])

        # ---- output projection -------------------------------------------
        for t in range(NT):
            for n2 in range(2):
                acc = ps.tile([128, 512], F32, tag="ps", name="acco")
                for c in range(GD // 128):
                    nc.tensor.matmul(
                        acc[:],
                        ot[:, c, t * 128 : (t + 1) * 128],
                        wo[:, c, n2 * 512 : (n2 + 1) * 512],
                        start=(c == 0),
                        stop=(c == GD // 128 - 1),
                    )
                ob = opool.tile([128, 512], F32, tag="ob")
                nc.vector.tensor_copy(ob[:], acc[:])
                nc.sync.dma_start(
                    out_d[t * 128 : (t + 1) * 128, n2 * 512 : (n2 + 1) * 512], ob[:]
                )


_NC_CACHE: dict = {}


def _get_kernel(mode: str, reps: int = 1):
    key = (mode, reps)
    if key not in _NC_CACHE:
        _NC_CACHE[key] = build_attention_kernel(mode, reps)
    return _NC_CACHE[key]


def make_in_maps(x, mask, Wqkv, Wout):
    tril = np.tril(np.ones((T, T), dtype=np.int32))
    m = np.asarray(mask[0, 0])
    if np.array_equal(m, tril):
        mode = "causal"
    elif np.all(m == 1):
        mode = "dense"
    else:
        mode = "masked"

    triu128 = np.triu(np.ones((128, 128), dtype=np.float32)).astype(NPBF16)
    in_maps = []
    for c in range(8):
        b, g = c // 2, c % 2
        im = {
            "xT": np.ascontiguousarray(x[b].T).astype(NPBF16),
            "wq": Wqkv[:, g * GD : (g + 1) * GD].astype(NPBF16),
            "wk": (Wqkv[:, D + g * GD : D + (g + 1) * GD] * SCALE).astype(NPBF16),
            "wv": Wqkv[:, 2 * D + g * GD : 2 * D + (g + 1) * GD].astype(NPBF16),
            "wout": Wout[g * GD : (g + 1) * GD, :].astype(NPBF16),
            "triu": triu128,
        }
        if mode == "masked":
            im["m01T"] = np.ascontiguousarray(m.T).astype(NPBF16)
        in_maps.append(im)
    return mode, in_maps


def kernel(x, mask, Wqkv, Wout):
    x = np.asarray(x)
    mask = np.asarray(mask)
    Wqkv = np.asarray(Wqkv)
    Wout = np.asarray(Wout)
    mode, in_maps = make_in_maps(x, mask, Wqkv, Wout)
    nc = _get_kernel(mode)
    res = run_bass_kernel_spmd(nc, in_maps, core_ids=list(range(8)))
    out = np.zeros((B, T, D), dtype=np.float32)
    for c in range(8):
        out[c // 2] += res.results[c]["out"]
    return out


# revision 13
# speedup vs baseline: 146.6299x; 1.6032x over previous
"""Multi-head causal attention (B=4, T=2048, D=1024, H=16, Dh=64) on 8 trn2 cores.

Sharding: core c -> (batch b = c//2, head-group g = c%2 covering heads 8g..8g+7).
Each core computes QKV projection for its batch/head-group, causal attention,
and a partial output projection over its 512 head-dims.  Host sums the two
partial outputs per batch.

Device-side dataflow per core (all matmul inputs bf16, accumulation fp32):
  xT [1024, 2048]  (x[b] transposed on host)
  qT, kT [512, 2048]   (projection emitted transposed: heads stacked on partitions,
                        pairs of heads per 128-partition block)
  v  [2048, 8, 65]     (natural layout, 65th column = 1.0 so the PV matmul also
                        produces the softmax denominator as output row 64)
  S^T [k, q] blocks via PE row-tiling: two K=64 head-matmuls run concurrently
  P = exp(S^T) on ScalarE (no max subtraction: |scores| <~ 8 by construction)
  causal: strictly-upper blocks skipped, diagonal 128x128 blocks masked with a
  multiplicative triu constant on VectorE
  out^T_aug [65, q] = v_aug^T @ P^T accumulated in PSUM; row 64 = sum(exp)
  normalize: reciprocal of row 64, DMA-broadcast across 64 partitions, multiply
  out-projection: OT [512, 2048] as lhsT against Wout shard, fp32 partial out
"""

import sys

if "/opt/trn_rl_repo" not in sys.path:
    sys.path.insert(0, "/opt/trn_rl_repo")

from contextlib import ExitStack

import ml_dtypes
import numpy as np

import concourse.bass as bass
import concourse.bacc as bacc
import concourse.mybir as mybir
import concourse.tile as tile
from concourse.bass_utils import run_bass_kernel_spmd

BF16 = mybir.dt.bfloat16
F32 = mybir.dt.float32
NPBF16 = ml_dtypes.bfloat16

B, T, D = 4, 2048, 1024
H, DH = 16, 64
HPG = 8          # heads per group (per core)
GD = HPG * DH    # 512 head-dims per core
NT = T // 128    # 16 t-blocks
NC = D // 128    # 8 model-dim chunks
NQ = T // 512    # 4 q-spans
SCALE = 1.0 / 8.0


def build_attention_kernel(mode: str = "causal", reps: int = 1, phases: str = "all"):
    """mode: 'causal' (tril mask), 'dense' (all-ones mask), 'masked' (arbitrary).

    reps > 1 wraps the compute body in a hardware For_i loop (for timing the
    kernel body without host dispatch overhead).  phases: 'all' | 'proj' |
    'att' | 'noexp' (timing experiments)."""
    nc = bacc.Bacc("TRN2", target_bir_lowering=False)

    xT_d = nc.dram_tensor("xT", [D, T], BF16, kind="ExternalInput")
    wq_d = nc.dram_tensor("wq", [D, GD], BF16, kind="ExternalInput")
    wk_d = nc.dram_tensor("wk", [D, GD], BF16, kind="ExternalInput")
    wv_d = nc.dram_tensor("wv", [D, GD], BF16, kind="ExternalInput")
    wo_d = nc.dram_tensor("wout", [GD, D], BF16, kind="ExternalInput")
    triu_d = nc.dram_tensor("triu", [128, 128], BF16, kind="ExternalInput")
    if mode == "masked":
        m01_d = nc.dram_tensor("m01T", [T, T], BF16, kind="ExternalInput")
    out_d = nc.dram_tensor("out", [T, D], F32, kind="ExternalOutput")

    with tile.TileContext(nc) as tc, ExitStack() as ctx:
        const = ctx.enter_context(tc.tile_pool(name="const", bufs=1))
        ppool = ctx.enter_context(tc.tile_pool(name="ppool", bufs=6))
        rpool = ctx.enter_context(tc.tile_pool(name="rpool", bufs=4))
        bpool = ctx.enter_context(tc.tile_pool(name="bpool", bufs=4))
        opool = ctx.enter_context(tc.tile_pool(name="opool", bufs=4))
        mpool = ctx.enter_context(tc.tile_pool(name="mpool", bufs=4))
        ps = ctx.enter_context(
            tc.tile_pool(name="ps", bufs=4, space=bass.MemorySpace.PSUM)
        )
        ps2 = ctx.enter_context(
            tc.tile_pool(name="ps2", bufs=2, space=bass.MemorySpace.PSUM)
        )

        # ---- load inputs --------------------------------------------------
        xT = const.tile([128, NC, T], BF16)
        wq = const.tile([128, NC, GD], BF16)
        wk = const.tile([128, NC, GD], BF16)
        wv = const.tile([128, NC, GD], BF16)
        for c in range(NC):
            nc.sync.dma_start(xT[:, c, :], xT_d[c * 128 : (c + 1) * 128, :])
            nc.sync.dma_start(wq[:, c, :], wq_d[c * 128 : (c + 1) * 128, :])
            nc.sync.dma_start(wk[:, c, :], wk_d[c * 128 : (c + 1) * 128, :])
            nc.sync.dma_start(wv[:, c, :], wv_d[c * 128 : (c + 1) * 128, :])
        wo = const.tile([128, GD // 128, D], BF16)
        for c in range(GD // 128):
            nc.sync.dma_start(wo[:, c, :], wo_d[c * 128 : (c + 1) * 128, :])
        triu = const.tile([128, 128], BF16)
        nc.sync.dma_start(triu[:], triu_d[:])

        qT = const.tile([128, GD // 128, T], BF16)
        kT = const.tile([128, GD // 128, T], BF16)
        v = const.tile([128, NT, HPG, DH + 1], BF16)
        ot = const.tile([128, GD // 128, T], BF16)

        nc.vector.memset(v[:, :, :, DH : DH + 1], 1.0)

        def body():
            _body(nc, tc, mode, ps, ps2, ppool, rpool, bpool, opool, mpool,
                  xT, wq, wk, wv, wo, triu, qT, kT, v, ot,
                  m01_d if mode == "masked" else None, out_d, phases)

        if reps > 1:
            with tc.For_i(0, reps, 1):
                body()
        else:
            body()

    nc.compile()
    return nc


def _body(nc, tc, mode, ps, ps2, ppool, rpool, bpool, opool, mpool,
          xT, wq, wk, wv, wo, triu, qT, kT, v, ot, m01_d, out_d, phases="all"):
    if True:
        # ---- qkv projection ----------------------------------------------
        def proj_qk(dst, w, j):
            # dst[:, j, :] <- (w[:, j*128:(j+1)*128]).T @ xT   -> [128, T]
            for np2 in range(NQ // 2):
                acc = [ps.tile([128, 512], F32, tag="ps", name=f"acc{n}") for n in range(2)]
                for c in range(NC):
                    lhsT = w[:, c, j * 128 : (j + 1) * 128]
                    for ni in range(2):
                        n = 2 * np2 + ni
                        nc.tensor.matmul(
                            acc[ni][:],
                            lhsT,
                            xT[:, c, n * 512 : (n + 1) * 512],
                            start=(c == 0),
                            stop=(c == NC - 1),
                        )
                for ni in range(2):
                    n = 2 * np2 + ni
                    nc.vector.tensor_copy(dst[:, j, n * 512 : (n + 1) * 512], acc[ni][:])

        def proj_v(t):
            acc = ps.tile([128, 512], F32, tag="ps", name="accv")
            for c in range(NC):
                nc.tensor.matmul(
                    acc[:],
                    xT[:, c, t * 128 : (t + 1) * 128],
                    wv[:, c, :],
                    start=(c == 0),
                    stop=(c == NC - 1),
                )
            nc.vector.tensor_copy(
                v[:, t, :, 0:DH], acc[:].rearrange("p (h e) -> p h e", e=DH)
            )

        def att(qi, j):
            if True:
                q0 = qi * 512
                nki = 4 * qi + 4 if mode == "causal" else NT
                pv = [ps.tile([128, 512], F32, tag="ps", name=f"pv{h}") for h in range(2)]
                for ki in range(nki):
                    d = ki - 4 * qi  # >=0: diagonal band (causal mode only)
                    lo = max(d, 0) * 128 if mode == "causal" else 0
                    if mode == "masked":
                        m01 = mpool.tile([128, 512], BF16, tag="m01")
                        nc.sync.dma_start(
                            m01[:],
                            m01_d[ki * 128 : (ki + 1) * 128, q0 : q0 + 512],
                        )
                    # both heads' scores in one 2-bank PSUM tile -> one exp
                    s2 = ps2.tile([128, 1024], F32, tag="ps2", name="s2")
                    for hh in range(2):
                        nc.tensor.matmul(
                            s2[:, hh * 512 + lo : hh * 512 + 512],
                            kT[hh * 64 : (hh + 1) * 64, j, ki * 128 : (ki + 1) * 128],
                            qT[hh * 64 : (hh + 1) * 64, j, q0 + lo : q0 + 512],
                            start=True,
                            stop=True,
                        )
                    p2 = ppool.tile([128, 1024], BF16, tag="p")
                    if lo == 0:
                        nc.scalar.activation(
                            p2[:], s2[:], mybir.ActivationFunctionType.Exp
                        )
                    else:
                        sv = s2.rearrange("p (h w) -> p h w", h=2)[:, :, lo:512]
                        pw = p2.rearrange("p (h w) -> p h w", h=2)[:, :, lo:512]
                        nc.scalar.activation(
                            pw, sv, mybir.ActivationFunctionType.Exp
                        )
                    if mode == "causal" and 0 <= d:
                        pd = p2.rearrange("p (h w) -> p h w", h=2)[:, :, lo : lo + 128]
                        triu2 = bass.AP(
                            tensor=triu.tensor,
                            offset=triu.offset,
                            ap=[list(triu.ap[0]), [0, 2], list(triu.ap[1])],
                        )
                        nc.vector.tensor_mul(pd, pd, triu2)
                    elif mode == "masked":
                        pm = p2.rearrange("p (h w) -> p h w", h=2)
                        m2 = bass.AP(
                            tensor=m01.tensor,
                            offset=m01.offset,
                            ap=[list(m01.ap[0]), [0, 2], list(m01.ap[1])],
                        )
                        nc.vector.tensor_mul(pm, pm, m2)
                    for hh in range(2):
                        nc.tensor.matmul(
                            pv[hh][0:65, lo:512],
                            v[:, ki, 2 * j + hh, :],
                            p2[:, hh * 512 + lo : hh * 512 + 512],
                            start=(ki == 0),
                            stop=(ki == nki - 1),
                        )
                # normalize and store into ot.  Evacuate PSUM to SBUF first so
                # the accumulator banks free up quickly for the next pair's QK.
                for hh in range(2):
                    stage = rpool.tile([65, 512], F32, tag="stage")
                    nc.scalar.copy(stage[:], pv[hh][:65, :])
                    recip = rpool.tile([1, 512], F32, tag="recip")
                    nc.vector.reciprocal(recip[:], stage[64:65, :])
                    bc = bpool.tile([64, 512], F32, tag="bc")
                    nc.gpsimd.partition_broadcast(bc[:], recip[:])
                    if hh == 0:
                        nc.vector.tensor_mul(
                            ot[0:64, j, q0 : q0 + 512], stage[0:64, :], bc[:]
                        )
                    else:
                        otn = opool.tile([64, 512], BF16, tag="otn")
                        nc.vector.tensor_mul(otn[:], stage[0:64, :], bc[:])
                        nc.sync.dma_start(ot[64:128, j, q0 : q0 + 512], otn[:])

        # ---- output projection -------------------------------------------
        # one LDWEIGHTS per (t, c): both 512-wide output halves share lhsT
        def outproj(t):
            acc2 = [ps.tile([128, 512], F32, tag="ps", name=f"acco{n}") for n in range(2)]
            for c in range(GD // 128):
                lhsT = ot[:, c, t * 128 : (t + 1) * 128]
                for n2 in range(2):
                    nc.tensor.matmul(
                        acc2[n2][:],
                        lhsT,
                        wo[:, c, n2 * 512 : (n2 + 1) * 512],
                        start=(c == 0),
                        stop=(c == GD // 128 - 1),
                    )
            for n2 in range(2):
                ob = opool.tile([128, 512], F32, tag="ob")
                nc.vector.tensor_copy(ob[:], acc2[n2][:])
                nc.sync.dma_start(
                    out_d[t * 128 : (t + 1) * 128, n2 * 512 : (n2 + 1) * 512], ob[:]
                )

        # ---- software-pipelined schedule ---------------------------------
        # Interleave projections, attention, and out-projection in program
        # order so PE-heavy and ACT-heavy work overlaps (shared PSUM pool
        # slots are granted in program order).
        do_proj = phases in ("all", "proj", "noexp")
        do_att = phases in ("all", "att", "noexp")
        do_out = phases in ("all", "noexp")
        if do_proj:
            proj_qk(qT, wq, 0)
            proj_qk(kT, wk, 0)
            for t in range(4):
                proj_v(t)
        if do_att:
            att(0, 0)
        for j in range(1, GD // 128):
            if do_proj:
                proj_qk(qT, wq, j)
                proj_qk(kT, wk, j)
            if do_att:
                att(0, j)
        if do_out:
            for t in range(4):
                outproj(t)
        for qi in range(1, NQ):
            if do_proj:
                for t in range(4 * qi, 4 * qi + 4):
                    proj_v(t)
            if do_att:
                for j in range(GD // 128):
                    att(qi, j)
            if do_out:
                for t in range(4 * qi, 4 * qi + 4):
                    outproj(t)


_NC_CACHE: dict = {}


def _get_kernel(mode: str, reps: int = 1, phases: str = "all"):
    key = (mode, reps, phases)
    if key not in _NC_CACHE:
        _NC_CACHE[key] = build_attention_kernel(mode, reps, phases)
    return _NC_CACHE[key]


def make_in_maps(x, mask, Wqkv, Wout):
    tril = np.tril(np.ones((T, T), dtype=np.int32))
    m = np.asarray(mask[0, 0])
    if np.array_equal(m, tril):
        mode = "causal"
    elif np.all(m == 1):
        mode = "dense"
    else:
        mode = "masked"

    triu128 = np.triu(np.ones((128, 128), dtype=np.float32)).astype(NPBF16)
    in_maps = []
    for c in range(8):
        b, g = c // 2, c % 2
        im = {
            "xT": np.ascontiguousarray(x[b].T).astype(NPBF16),
            "wq": Wqkv[:, g * GD : (g + 1) * GD].astype(NPBF16),
            "wk": (Wqkv[:, D + g * GD : D + (g + 1) * GD] * SCALE).astype(NPBF16),
            "wv": Wqkv[:, 2 * D + g * GD : 2 * D + (g + 1) * GD].astype(NPBF16),
            "wout": Wout[g * GD : (g + 1) * GD, :].astype(NPBF16),
            "triu": triu128,
        }
        if mode == "masked":
            im["m01T"] = np.ascontiguousarray(m.T).astype(NPBF16)
        in_maps.append(im)
    return mode, in_maps


def kernel(x, mask, Wqkv, Wout):
    x = np.asarray(x)
    mask = np.asarray(mask)
    Wqkv = np.asarray(Wqkv)
    Wout = np.asarray(Wout)
    mode, in_maps = make_in_maps(x, mask, Wqkv, Wout)
    nc = _get_kernel(mode)
    res = run_bass_kernel_spmd(nc, in_maps, core_ids=list(range(8)))
    out = np.zeros((B, T, D), dtype=np.float32)
    for c in range(8):
        out[c // 2] += res.results[c]["out"]
    return out
